# revision 50
# baseline (speedup 1.0000x reference)
"""Sparse-attention kernel for Trainium2 (8 NeuronCores, data-parallel).

reference (per batch b):
    h     = relu(k @ w1 + q @ w2 + bias)          [L, D]
    alpha = h @ w0.T                               [L, S]
    alpha = where(mask == 0, -1e9, alpha)
    alpha = softmax(alpha, axis=L)
    out   = alpha.T @ v                            [S, D]

Device mapping (per core, 512 batches, ~194 us vs 312.5 us baseline):
  - host pre-transposes q,k to [D, Bc*L] so the D-contraction has D on
    partitions; weights are replicated.
  - softmax over L has no max-subtraction (alpha is O(3), masked lanes
    multiply by 0 after exp — identical to exp(-1e9)=0 in the reference).
  - dtypes: q,k fp8-e3m4 (PE accepts bf16-stationary x e3m4-moving),
    v/weights/h/e bf16, mask fp8-e4m3 (0/1 exact), PSUM fp32.
    Scale-relative max error 1.32e-2, bit-identical to the numpy
    simulation of the same dtype pipeline (fixed seed).
  - e = exp(alpha)*mask is packed block-diagonally per batch-pair (the
    host ships the mask at double width with off-diagonal zeros; e0 is
    read twice per row via a stride-0 AP) so one K=100 matmul covers
    both batches of a pair.
  - step 4 runs transposed: num^T = v.T @ blockdiag(e) lands as
    [128, 100] tiles (full-partition stores); a 1-column matmul per
    pair emits the softmax denominator and the division happens on the
    host — no reciprocal/divide ops on device.
  - DMAs move 2 groups each (mask 4), spread across the sync/gpsimd
    queues; the first two pair-loads go group-by-group and data loads
    are issued before the weight loads so compute starts early.
  - emission is phase-major over super-groups of 4 so the PE gets
    dense matmul bursts; step 4 trails by one super-group.  (Coarser
    DMAs/sems, shorter trailing, shared den tiles, and PSUM-buffer
    rebalances were all measured and all regress — this structure is a
    sharp local optimum under the Tile scheduler.)
"""
import os
import sys

for p in ("/opt/trn_rl_repo", "/root/.axon_site", "/root/.axon_site/_ro/trn_rl_repo"):
    if os.path.isdir(p) and p not in sys.path:
        sys.path.append(p)

import numpy as np
import ml_dtypes

import concourse.bass as bass
import concourse.tile as tile
from concourse import mybir
from concourse.bass_utils import run_bass_kernel_spmd

# ---------------------------------------------------------------------------
# Workaround for this walrus build's limit of ONE sync-wait per instruction:
# hoist extra waits onto same-engine NoOps inserted just before.
_wsplit_counter = [0]


def _split_multi_waits(nc):
    for fn in nc.m.functions:
        for bb in fn.blocks:
            out = []
            changed = False
            for inst in bb.instructions:
                si = inst.sync_info
                if si is not None and len(si.on_wait) > 1:
                    waits = list(si.on_wait)
                    for w in waits[:-1]:
                        _wsplit_counter[0] += 1
                        nop = mybir.InstNoOp(
                            name=f"I-wsplit-{_wsplit_counter[0]}",
                            ins=[],
                            outs=[],
                            engine=inst.engine,
                        )
                        nop.sync_info = mybir.SyncInfo(on_wait=[w], on_update=[])
                        out.append(nop)
                    inst.sync_info = mybir.SyncInfo(
                        on_wait=[waits[-1]], on_update=list(si.on_update)
                    )
                    changed = True
                out.append(inst)
            if changed:
                bb.instructions = out


# ---------------------------------------------------------------------------
B, L, D, S = 4096, 50, 256, 50
M = 8                 # cores
Bc = B // M           # batches per core
G = 8                 # batches per group
NG = Bc // G          # groups per core (64)
TOK = G * L           # tokens per group (400)
VA_W = D + 2          # v augmented with [1, 0] columns
NGL = NG // 2         # 2-group load/store batches (32)
NGM = NG // 4         # 4-group mask load batches (16)

f32 = mybir.dt.float32
bf16 = mybir.dt.bfloat16
fp8 = mybir.dt.float8e4
fp8e3 = mybir.dt.float8e3
AF = mybir.ActivationFunctionType

_cache = {}


def _build():
    if "nc" in _cache:
        return _cache["nc"]
    nc = bass.Bass("TRN2", target_bir_lowering=False, debug=False)
    qt_d = nc.dram_tensor("qt", [128, NGL, 2, 2, TOK], fp8e3, kind="ExternalInput").ap()
    kt_d = nc.dram_tensor("kt", [128, NGL, 2, 2, TOK], fp8e3, kind="ExternalInput").ap()
    va_d = nc.dram_tensor("va", [100, NGL, 2, 4, VA_W], bf16, kind="ExternalInput").ap()
    mk_d = nc.dram_tensor("mk", [100, NGM, 4, 4, 2 * S], fp8, kind="ExternalInput").ap()
    w1_d = nc.dram_tensor("w1", [D, D], bf16, kind="ExternalInput").ap()
    w2_d = nc.dram_tensor("w2", [D, D], bf16, kind="ExternalInput").ap()
    w0t_d = nc.dram_tensor("w0t", [D, S], bf16, kind="ExternalInput").ap()
    bias_d = nc.dram_tensor("bias", [D, 1], f32, kind="ExternalInput").ap()
    # unnormalized numerator, transposed: [d-chunk partition, load-group,
    # group-slot, chunk, pair, (parity, s)]
    out_d = nc.dram_tensor(
        "out", [128, NGL, 2, 2, 4, 100], bf16, kind="ExternalOutput"
    ).ap()
    den_d = nc.dram_tensor("den", [100, 8, 8, 4], bf16, kind="ExternalOutput").ap()

    with tile.TileContext(nc) as tc:
        with (
            tc.tile_pool(name="singles", bufs=1) as singles,
            tc.tile_pool(name="qk", bufs=5) as qk,
            tc.tile_pool(name="htp", bufs=5) as htp,
            tc.tile_pool(name="vm", bufs=5) as vm,
            tc.tile_pool(name="ep", bufs=8) as ep,
            tc.tile_pool(name="osb", bufs=4) as osb,
            tc.tile_pool(name="denp", bufs=2) as denp,
            tc.tile_pool(name="ht_ps", bufs=4, space="PSUM") as ht_ps,
            tc.tile_pool(name="al_ps", bufs=2, space="PSUM") as al_ps,
            tc.tile_pool(name="o_ps", bufs=2, space="PSUM") as o_ps,  # [128,2,101] f32

        ):
            w1_t = singles.tile([128, 2, D], bf16)
            w2_t = singles.tile([128, 2, D], bf16)
            w0_t = singles.tile([128, 2, S], bf16)
            b_t = singles.tile([128, 2, 1], f32)

            def emit_pair_loads(gl):
                # loads covering groups 2*gl, 2*gl+1; the very first pair
                # loads group-by-group so compute starts on group 0 asap
                qt_t = qk.tile([128, 2, 2, TOK], fp8e3, tag="qt")
                kt_t = qk.tile([128, 2, 2, TOK], fp8e3, tag="kt")
                va_t = vm.tile([100, 2, 4, VA_W], bf16, tag="va")
                if gl <= 1:
                    for j in range(2):
                        nc.sync.dma_start(qt_t[:, j], qt_d[:, gl, j])
                        nc.gpsimd.dma_start(kt_t[:, j], kt_d[:, gl, j])
                        nc.gpsimd.dma_start(va_t[:, j], va_d[:, gl, j])
                else:
                    nc.sync.dma_start(qt_t[:], qt_d[:, gl])
                    nc.gpsimd.dma_start(kt_t[:], kt_d[:, gl])
                    if gl % 2 == 0:
                        nc.gpsimd.dma_start(va_t[:], va_d[:, gl])
                    else:
                        nc.sync.dma_start(va_t[:], va_d[:, gl])
                return qt_t, kt_t, va_t

            def emit_mask_load(gm):
                mk_t = vm.tile([100, 4, 4, 2 * S], fp8, tag="mk")
                nc.sync.dma_start(mk_t[:], mk_d[:, gm])
                return mk_t

            def emit_s1(qt_t, kt_t, sl):
                # step 1: ht = relu(w1.T @ kt + w2.T @ qt + bias)  [D, TOK]
                ht_t = htp.tile([128, 2, TOK], bf16)
                for co in range(2):
                    hp = ht_ps.tile([128, TOK], f32)
                    cs = slice(co * 128, (co + 1) * 128)
                    nc.tensor.matmul(hp[:], w1_t[:, 0, cs], kt_t[:, sl, 0, :], start=True, stop=False)
                    nc.tensor.matmul(hp[:], w1_t[:, 1, cs], kt_t[:, sl, 1, :], start=False, stop=False)
                    nc.tensor.matmul(hp[:], w2_t[:, 0, cs], qt_t[:, sl, 0, :], start=False, stop=False)
                    nc.tensor.matmul(hp[:], w2_t[:, 1, cs], qt_t[:, sl, 1, :], start=False, stop=True)
                    if co == 0:
                        nc.scalar.activation(
                            ht_t[:, co, :], hp[:], AF.Relu, bias=b_t[:, co, :]
                        )
                    else:
                        nc.vector.tensor_scalar(
                            ht_t[:, co, :], hp[:], b_t[:, co, :], 0.0,
                            mybir.AluOpType.add, mybir.AluOpType.max,
                        )

                return ht_t

            def emit_s3(g, ht_t, mk_t):
                # step 2: alpha = ht.T @ w0t, one M=100 matmul pair per
                # token-pair (100 contiguous tokens)
                ap_t = al_ps.tile([100, 4, S], f32)
                for p in range(4):
                    bc = slice(p * 100, (p + 1) * 100)
                    nc.tensor.matmul(
                        ap_t[:, p, :], ht_t[:, 0, bc], w0_t[:, 0, :],
                        start=True, stop=False,
                    )
                    nc.tensor.matmul(
                        ap_t[:, p, :], ht_t[:, 1, bc], w0_t[:, 1, :],
                        start=False, stop=True,
                    )

                # step 3: e = exp(alpha) * mask, packed block-diagonally per
                # pair (even batch rows 0:50 x cols 0:50, odd batch rows
                # 50:100 x cols 50:100, zeros elsewhere) so one K=100 matmul
                # computes both batches of a pair.  The mask tensor is
                # host-prepared at double width with the off-diagonal blocks
                # zeroed; e0 is read twice per row via a stride-0 AP.
                e0_t = ep.tile([100, 4, S], bf16, tag="e0")
                nc.scalar.activation(e0_t[:], ap_t[:], AF.Exp)
                e_t = ep.tile([100, 4, 2 * S], bf16, tag="e")
                e0_ap = e0_t[:]
                e0_bcast = bass.AP(
                    tensor=e0_ap.tensor,
                    offset=e0_ap.offset,
                    ap=[e0_ap.ap[0], e0_ap.ap[1], [0, 2], e0_ap.ap[2]],
                )
                nc.vector.tensor_mul(
                    e_t[:].rearrange("l p (r s) -> l p r s", r=2),
                    e0_bcast,
                    mk_t[:, g % 4].rearrange("l p (r s) -> l p r s", r=2),
                )
                return e_t

            def emit_s5(g, e_t, va_t, o_t, den_t):
                # step 4 (transposed): num^T = v.T @ blockdiag(e)  [D, 2S]
                # per pair, plus a 1-column matmul for the softmax
                # denominator; the division happens on the host.
                sl = g % 2
                for pp in range(2):
                    # two pairs share one PSUM bank tile; one evacuation op
                    # and one denominator copy cover both
                    op_t = o_ps.tile([128, 2, 2, 101], f32)
                    for j in range(2):
                        p = 2 * pp + j
                        nc.tensor.matmul(
                            op_t[:, j, 0, 0:100], va_t[:, sl, p, 0:128],
                            e_t[:, p, :], start=True, stop=True,
                        )
                        nc.tensor.matmul(
                            op_t[:, j, 1, 0:100], va_t[:, sl, p, 128:256],
                            e_t[:, p, :], start=True, stop=True,
                        )
                        nc.tensor.matmul(
                            op_t[0:100, j, 0, 100:101], e_t[:, p, :],
                            va_t[:, sl, p, D : D + 1],
                            start=True, stop=True,
                        )
                    src_ap = op_t[:, :, :, 0:100].rearrange("d j c f -> d c j f")
                    if pp == 0:
                        nc.scalar.activation(
                            o_t[:, sl, :, 2 * pp : 2 * pp + 2, :], src_ap, AF.Copy
                        )
                    else:
                        nc.vector.tensor_copy(
                            o_t[:, sl, :, 2 * pp : 2 * pp + 2, :], src_ap
                        )
                    nc.vector.tensor_copy(
                        den_t[:, g % 8, 2 * pp : 2 * pp + 2],
                        op_t[0:100, :, 0, 100:101].rearrange("p j o -> p (j o)"),
                    )

            # Phase-major super-groups: batch each phase across SG groups
            # so the PE gets long dense matmul bursts and cross-engine
            # handoffs amortize.  Step 5 runs one super-group behind so its
            # e/v inputs are long since ready.
            SG = 4
            state = {"o_t": None, "den_t": None}

            def run_s5(g, e_t, va_t):
                if g % 2 == 0:
                    state["o_t"] = osb.tile([128, 2, 2, 4, 100], bf16, name="o_t")
                if g % 8 == 0:
                    state["den_t"] = denp.tile([100, 8, 4], bf16, name="den_t")
                emit_s5(g, e_t, va_t, state["o_t"], state["den_t"])
                if g % 2 == 1:
                    nc.scalar.dma_start(out_d[:, g // 2], state["o_t"][:])
                if g % 8 == 7:
                    nc.sync.dma_start(den_d[:, g // 8], state["den_t"][:])

            prev = []
            mk_t = None
            for s in range(NG // SG):
                gs = list(range(s * SG, (s + 1) * SG))
                pl0 = emit_pair_loads(s * 2)
                pl1 = emit_pair_loads(s * 2 + 1)
                mk_t = emit_mask_load(s)
                if s == 0:
                    # weights issue after the first data loads so compute
                    # on group 0 starts as early as possible
                    nc.sync.dma_start(
                        w1_t[:], w1_d.rearrange("(c p) n -> p c n", p=128)
                    )
                    nc.sync.dma_start(
                        w2_t[:], w2_d.rearrange("(c p) n -> p c n", p=128)
                    )
                    nc.sync.dma_start(
                        w0_t[:], w0t_d.rearrange("(c p) s -> p c s", p=128)
                    )
                    nc.sync.dma_start(
                        b_t[:], bias_d.rearrange("(c p) o -> p c o", p=128)
                    )
                loads = [pl0, pl0, pl1, pl1]
                hts = [
                    emit_s1(lt[0], lt[1], g % 2) for g, lt in zip(gs, loads)
                ]
                cur = []
                for g, lt, ht_t in zip(gs, loads, hts):
                    e_t = emit_s3(g, ht_t, mk_t)
                    cur.append((g, e_t, lt[2]))
                for g, e_t, va_t in prev:
                    run_s5(g, e_t, va_t)
                prev = cur
            for g, e_t, va_t in prev:
                run_s5(g, e_t, va_t)

    _split_multi_waits(nc)
    _cache["nc"] = nc
    return nc


def _make_in_maps(q, k, v, mask):
    in_maps = []
    for c in range(M):
        sl = slice(c * Bc, (c + 1) * Bc)
        # [128, NGL, 2, 2, TOK]: partition-contiguous group slices
        qs = np.ascontiguousarray(
            q[sl].reshape(NG, TOK, 2, 128).transpose(3, 0, 2, 1)
        ).astype(ml_dtypes.float8_e3m4).reshape(128, NGL, 2, 2, TOK)
        ks = np.ascontiguousarray(
            k[sl].reshape(NG, TOK, 2, 128).transpose(3, 0, 2, 1)
        ).astype(ml_dtypes.float8_e3m4).reshape(128, NGL, 2, 2, TOK)
        va = np.zeros((100, NG, 4, VA_W), dtype=ml_dtypes.bfloat16)
        va[:, :, :, :D] = v[sl].reshape(NG, 4, 100, D).transpose(2, 0, 1, 3).astype(ml_dtypes.bfloat16)
        va[:, :, :, D] = 1.0
        va = va.reshape(100, NGL, 2, 4, VA_W)
        # double-width block-diagonal mask in fp8 (0/1 are exact)
        m5 = mask[sl].reshape(NG, 4, 2, 50, S).transpose(3, 0, 1, 2, 4)
        mk = np.zeros((100, NG, 4, 2, S), dtype=ml_dtypes.float8_e4m3)
        mk[0:50, :, :, 0, :] = m5[:, :, :, 0, :]
        mk[50:100, :, :, 1, :] = m5[:, :, :, 1, :]
        mk = mk.reshape(100, NGM, 4, 4, 2 * S)
        in_maps.append({"qt": qs, "kt": ks, "va": va, "mk": mk})
    return in_maps


def _run(q, k, v, mask, attn_w0, attn_w1, attn_w2, attn_bias, **run_kwargs):
    nc = _build()
    w1 = np.ascontiguousarray(attn_w1).astype(ml_dtypes.bfloat16)
    w2 = np.ascontiguousarray(attn_w2).astype(ml_dtypes.bfloat16)
    w0t = np.ascontiguousarray(np.asarray(attn_w0, dtype=np.float32).T).astype(
        ml_dtypes.bfloat16
    )
    bias = np.ascontiguousarray(
        np.asarray(attn_bias, dtype=np.float32).reshape(D, 1)
    )
    in_maps = _make_in_maps(
        np.asarray(q, dtype=np.float32),
        np.asarray(k, dtype=np.float32),
        np.asarray(v, dtype=np.float32),
        np.asarray(mask),
    )
    for im in in_maps:
        im.update({"w1": w1, "w2": w2, "w0t": w0t, "bias": bias})
    res = run_bass_kernel_spmd(nc, in_maps, core_ids=list(range(M)), **run_kwargs)
    parts = []
    for r in res.results:
        # num^T: [128 dp, NGL, slot, chunk, pair, (parity, s)]
        num = (
            r["out"]
            .astype(np.float32)
            .reshape(128, NGL, 2, 2, 4, 2, 50)
            .transpose(1, 2, 4, 5, 6, 3, 0)
            .reshape(Bc, S, D)
        )
        den = (
            r["den"]
            .astype(np.float32)
            .reshape(2, 50, 8, 8, 4)
            .transpose(2, 3, 4, 0, 1)
            .reshape(Bc, S)
        )
        parts.append(num / den[:, :, None])
    out = np.concatenate(parts, axis=0)
    return out, res


def kernel(q, k, v, mask, attn_w0, attn_w1, attn_w2, attn_bias):
    out, _ = _run(q, k, v, mask, attn_w0, attn_w1, attn_w2, attn_bias)
    return out
